# revision 31
# baseline (speedup 1.0000x reference)
"""MetapathAttentionLayer Trainium2 kernel (v3: packed node-metapath layout,
64-tile score super-regions, 32-tile pooling chunks).

Math (per node n):
    scores[n, m] = sum_d x[m, n, d] * W[d, m]
    att = softmax(relu(scores), axis=m)      (8 metapaths)
    out[n, :] = elu(sum_m att[n, m] * x[m, n, :])

Strategy: shard nodes across 8 cores (data parallel).  Per core, nodes are
packed so SBUF partition p = (node%32)*4 + metapath' holds one (node,
metapath) row of x, split into two halves (metapaths 0-3 / 4-7).

Per score-region of up to 64 tiles (2048 nodes):
  - scores: one DVE tensor_tensor multiply against a replicated-W pattern
    (2x mode), then a batched binary-tree reduction over d (each level is
    a single DVE op covering every tile in the region).
  - softmax over m: ACT relu+exp; per-node sums of the 8 metapaths via a
    PE matmul whose constant block stationary also replicates the sums to
    all four 32-partition blocks; DVE reciprocal + weight multiply.
Per 32-tile pooling chunk:
  - GPSIMD local_scatter packs attention weights into 32-wide stationaries
    (4 diagonals each); PE matmuls contract the (node, metapath) partition
    dim, accumulating both halves into PSUM.
  - elu(x) = relu(x) + exp(-relu(-x)) - 1: three ACT ops; the final
    (e2 + rl) add runs on PE (identity stationary, accumulated back into
    the dead pooling psum region); ACT applies the -1 and emits bf16.
The whole loop is software-pipelined over score-regions.
"""

from contextlib import ExitStack

import numpy as np
import ml_dtypes

import concourse.bass as bass
import concourse.tile as tile
from concourse import bacc, mybir, library_config
import concourse.bass_utils as bass_utils

F32 = mybir.dt.float32
BF16 = mybir.dt.bfloat16
I16 = mybir.dt.int16
ALU = mybir.AluOpType
ACTF = mybir.ActivationFunctionType

NMETA = 8
N = 100000
D = 128
NCORES = 8
NC_RAW = N // NCORES          # 12500 nodes per core
NC_PAD = 12800                # 400 tiles of 32 nodes
NTILES = NC_PAD // 32         # 400
SREG = 64                     # tiles per score region (2048 nodes)
CHUNK = 32                    # tiles per pooling/psum chunk (1024 nodes)
CPK_U16 = 4 * D + CHUNK       # packed const columns (u16)


def _sregion_sizes():
    """Tiles per score region: small ramp-in/out regions, 64-tile middle."""
    head = [16, 16]
    tail = [24, 16, 8]
    mid = NTILES - sum(head) - sum(tail)
    assert mid % SREG == 0
    return head + [SREG] * (mid // SREG) + tail


def kernel_body(tc, out_d, xa_d, xb_d, cpk_d):
    nc = tc.nc
    sizes = _sregion_sizes()
    starts = [sum(sizes[:i]) for i in range(len(sizes))]
    R = len(sizes)
    with ExitStack() as ctx:
        const = ctx.enter_context(tc.tile_pool(name="const", bufs=1))
        xpool = ctx.enter_context(tc.tile_pool(name="x", bufs=3))
        ppool = ctx.enter_context(tc.tile_pool(name="prod", bufs=1))
        tpool = ctx.enter_context(tc.tile_pool(name="tree", bufs=2))
        spool = ctx.enter_context(tc.tile_pool(name="smalls", bufs=3))
        scat = ctx.enter_context(tc.tile_pool(name="scat", bufs=4))
        epool = ctx.enter_context(tc.tile_pool(name="elu", bufs=4))
        opool = ctx.enter_context(tc.tile_pool(name="osb", bufs=4))
        psum = ctx.enter_context(tc.tile_pool(name="ps", bufs=3, space="PSUM"))
        psum_s = ctx.enter_context(tc.tile_pool(name="pss", bufs=2, space="PSUM"))

        # packed consts: [wba | wbb | blk4r | ident | sidx] as u16 columns
        cpk = const.tile([128, CPK_U16], mybir.dt.uint16)
        nc.sync.dma_start(cpk[:], cpk_d[:])
        wba = cpk[:, 0:D].bitcast(BF16)
        wbb = cpk[:, D:2 * D].bitcast(BF16)
        blk4r = cpk[:, 2 * D:3 * D].bitcast(BF16)
        ident = cpk[:, 3 * D:4 * D].bitcast(BF16)
        sidx = cpk[:, 4 * D:4 * D + CHUNK].bitcast(I16)
        neg1 = const.tile([128, 1], F32)
        nc.vector.memset(neg1[:], -1.0)
        nc.gpsimd.load_library(library_config.local_scatter)

        st = {}   # region -> dict of live tiles

        def stage_dma(r):
            nt = sizes[r]
            fw = nt * D
            d = {"nt": nt}
            for h, x_d in (("a", xa_d), ("b", xb_d)):
                xt = xpool.tile([128, SREG * D], BF16, tag=f"X{h}",
                                name=f"X{h}")
                nc.sync.dma_start(
                    xt[:, :fw],
                    x_d[:, starts[r]:starts[r] + nt, :].rearrange(
                        "p t d -> p (t d)"))
                d[f"X{h}"] = xt
            st[r] = d

        def stage_scores(r):
            """mult + tree + relu/exp + replicated Σe matmuls."""
            d = st[r]
            nt = d["nt"]
            fw = nt * D
            s = spool.tile([128, 2 * SREG], F32, tag="s")
            for hi, (h, wb) in enumerate((("a", wba), ("b", wbb))):
                P = ppool.tile([128, SREG * D], BF16, tag="P", name="P")
                nc.vector.tensor_tensor(
                    out=P[:, :fw].rearrange("p (t d) -> p t d", t=nt),
                    in0=d[f"X{h}"][:, :fw].rearrange("p (t d) -> p t d", t=nt),
                    in1=wb[:].unsqueeze(1).broadcast_to([128, nt, D]),
                    op=ALU.mult,
                )
                cur = P
                w = D // 2
                while w >= 2:
                    nxt = tpool.tile([128, SREG * w], BF16, tag=f"T{w}",
                                     name=f"T{w}")
                    cv = cur[:, :nt * 2 * w].rearrange(
                        "p (t d) -> p t d", t=nt)
                    nc.vector.tensor_tensor(
                        out=nxt[:, :nt * w].rearrange(
                            "p (t d) -> p t d", t=nt),
                        in0=cv[:, :, 0:w],
                        in1=cv[:, :, w:2 * w],
                        op=ALU.add,
                    )
                    cur = nxt
                    w //= 2
                cv = cur[:, :nt * 2].rearrange("p (t d) -> p t d", t=nt)
                nc.vector.tensor_tensor(
                    out=s[:, hi * nt:hi * nt + nt].unsqueeze(2),
                    in0=cv[:, :, 0:1],
                    in1=cv[:, :, 1:2],
                    op=ALU.add,
                )
            sr = spool.tile([128, 2 * SREG], BF16, tag="sr")
            e = spool.tile([128, 2 * SREG], BF16, tag="e")
            sm = psum_s.tile([128, SREG], F32, tag="sm")
            with tc.high_priority():
                nc.scalar.activation(sr[:, :2 * nt], s[:, :2 * nt], ACTF.Relu)
                nc.scalar.activation(e[:, :2 * nt], sr[:, :2 * nt], ACTF.Exp)
                # per-node sums replicated into all four 32-partition blocks
                nc.tensor.matmul(out=sm[:, 0:nt], lhsT=blk4r,
                                 rhs=e[:, 0:nt], start=True, stop=False)
                nc.tensor.matmul(out=sm[:, 0:nt], lhsT=blk4r,
                                 rhs=e[:, nt:2 * nt], start=False, stop=True)
            d["e"] = e
            d["sm"] = sm

        def stage_att(r):
            """recip + att weights + per-chunk scatters."""
            d = st[r]
            nt = d["nt"]
            e, sm = d["e"], d["sm"]
            inv = spool.tile([128, SREG], F32, tag="inv")
            with tc.high_priority():
                nc.vector.reciprocal(inv[:, :nt], sm[:, 0:nt])
            att = spool.tile([128, 2 * SREG], BF16, tag="att")
            nc.vector.scalar_tensor_tensor(
                out=att[:, :2 * nt].rearrange("p (h t) -> p h t", h=2),
                in0=e[:, :2 * nt].rearrange("p (h t) -> p h t", h=2),
                scalar=1.0,
                in1=inv[:, :nt].unsqueeze(1).broadcast_to([128, 2, nt]),
                op0=ALU.mult, op1=ALU.mult,
            )
            d["S"] = {}
            for c0 in range(0, nt, CHUNK):
                cn = min(CHUNK, nt - c0)
                for hi, h in enumerate(("a", "b")):
                    S = scat.tile([128, CHUNK * 32], BF16, tag=f"S{h}",
                                  name=f"S{h}")
                    nc.gpsimd.local_scatter(
                        S[:, :cn * 32],
                        att[:, hi * nt + c0:hi * nt + c0 + cn],
                        sidx[:, :cn], channels=128,
                        num_elems=cn * 32, num_idxs=cn)
                    d["S"][(c0, h)] = S

        def stage_pool_elu(r):
            """Per chunk: PE pooling, ACT elu, PE combine, ACT -1, DMA out."""
            d = st[r]
            nt = d["nt"]
            for c0 in range(0, nt, CHUNK):
                cn = min(CHUNK, nt - c0)
                nn = cn * 32
                pool_ps = psum.tile([128, CHUNK * 32], F32, tag="pool")
                for tt in range(cn):
                    po = 32 * (tt & 3)
                    co = D * (tt >> 2)
                    nc.tensor.matmul(
                        out=pool_ps[po:po + 32, co:co + D],
                        lhsT=d["S"][(c0, "a")][:, 32 * tt:32 * tt + 32],
                        rhs=d["Xa"][:, D * (c0 + tt):D * (c0 + tt) + D],
                        start=True, stop=False, tile_position=(0, po))
                    nc.tensor.matmul(
                        out=pool_ps[po:po + 32, co:co + D],
                        lhsT=d["S"][(c0, "b")][:, 32 * tt:32 * tt + 32],
                        rhs=d["Xb"][:, D * (c0 + tt):D * (c0 + tt) + D],
                        start=False, stop=True, tile_position=(0, po))
                rl = epool.tile([128, CHUNK * 32], BF16, tag="rl")
                nc.scalar.activation(rl[:, :nn], pool_ps[:, :nn], ACTF.Relu)
                t2 = epool.tile([128, CHUNK * 32], BF16, tag="t2")
                nc.scalar.activation(t2[:, :nn], pool_ps[:, :nn], ACTF.Relu,
                                     scale=-1.0)
                e2 = epool.tile([128, CHUNK * 32], BF16, tag="e2")
                nc.scalar.activation(e2[:, :nn], t2[:, :nn], ACTF.Exp,
                                     scale=-1.0)
                # cmb = e2 + rl on PE, into the dead pooling psum region
                for b0 in range(0, nn, 512):
                    bw = min(512, nn - b0)
                    nc.tensor.matmul(out=pool_ps[:, b0:b0 + bw], lhsT=ident,
                                     rhs=e2[:, b0:b0 + bw],
                                     start=True, stop=False)
                    nc.tensor.matmul(out=pool_ps[:, b0:b0 + bw], lhsT=ident,
                                     rhs=rl[:, b0:b0 + bw],
                                     start=False, stop=True)
                out_sb = opool.tile([128, CHUNK * 32], BF16, tag="osb")
                nc.scalar.activation(out_sb[:, :nn], pool_ps[:, :nn],
                                     ACTF.Identity, bias=neg1[:, 0:1])
                col = (starts[r] + c0) * 32
                nc.sync.dma_start(out_d[:, col:col + nn], out_sb[:, :nn])
            del st[r]

        # software pipeline over score regions
        for k in range(R + 2):
            if 2 <= k <= R + 1:
                stage_att(k - 2)
            if k < R:
                stage_dma(k)
            if 2 <= k <= R + 1:
                stage_pool_elu(k - 2)
            if 1 <= k <= R:
                stage_scores(k - 1)


def host_inputs(x_np, w_np):
    """Build per-core input maps from full fp32 inputs."""
    q = np.arange(128) >> 2          # node-in-tile per partition
    mi = np.arange(128) & 3          # metapath-within-half per partition

    wba = np.ascontiguousarray(w_np.T[mi, :]).astype(ml_dtypes.bfloat16)
    wbb = np.ascontiguousarray(w_np.T[4 + mi, :]).astype(ml_dtypes.bfloat16)
    blk4r = (q[:, None] == (np.arange(128)[None, :] >> 2)).astype(
        ml_dtypes.bfloat16)
    ident = np.eye(128, dtype=ml_dtypes.bfloat16)
    sidx = (32 * np.arange(CHUNK)[None, :] + q[:, None]).astype(np.int16)
    cpk = np.concatenate([
        wba.view(np.uint16), wbb.view(np.uint16), blk4r.view(np.uint16),
        ident.view(np.uint16), sidx.view(np.uint16)], axis=1)
    assert cpk.shape == (128, CPK_U16)

    in_maps = []
    for c in range(NCORES):
        xs = x_np[:, c * NC_RAW:(c + 1) * NC_RAW, :]
        xp = np.zeros((NMETA, NC_PAD, D), dtype=ml_dtypes.bfloat16)
        xp[:, :NC_RAW, :] = xs.astype(ml_dtypes.bfloat16)
        arr = xp.reshape(NMETA, NTILES, 32, D)
        # partition p = q*4 + mi  ->  [q, mi, t, d]
        xa = np.ascontiguousarray(
            arr[0:4].transpose(2, 0, 1, 3).reshape(128, NTILES, D))
        xb = np.ascontiguousarray(
            arr[4:8].transpose(2, 0, 1, 3).reshape(128, NTILES, D))
        in_maps.append({"xa": xa, "xb": xb, "cpk": cpk})
    return in_maps


def unshard(res):
    """Per-core [128, NC_PAD] bf16 psum-slot layout -> full [N, D] f32.

    Within each pooling chunk of cn tiles starting at tile t0:
    out[:, (t0 + 0..cn)*32] holds node 32*t0 + 32*(cblk*4 + pblk) + q at
    partition (32*pblk + q), column (cblk*D + d).
    """
    chunks = []
    for r, nt in enumerate(_sregion_sizes()):
        t0 = sum(_sregion_sizes()[:r])
        for c0 in range(0, nt, CHUNK):
            chunks.append((t0 + c0, min(CHUNK, nt - c0)))
    full = np.empty((NCORES, NC_RAW, D), dtype=np.float32)
    for c in range(NCORES):
        o = np.asarray(res.results[c]["out"]).astype(np.float32)
        parts = []
        for t0, cn in chunks:
            blk = o[:, t0 * 32:(t0 + cn) * 32].reshape(4, 32, cn // 4, D)
            parts.append(blk.transpose(2, 0, 1, 3).reshape(cn * 32, D))
        full[c] = np.concatenate(parts, axis=0)[:NC_RAW]
    return full.reshape(N, D)


_CACHE = {}


def build():
    if "nc" in _CACHE:
        return _CACHE["nc"]
    nc = bacc.Bacc("TRN2", target_bir_lowering=False, debug=False,
                   num_devices=NCORES)
    xa = nc.dram_tensor("xa", [128, NTILES, D], BF16, kind="ExternalInput").ap()
    xb = nc.dram_tensor("xb", [128, NTILES, D], BF16, kind="ExternalInput").ap()
    cpk = nc.dram_tensor("cpk", [128, CPK_U16], mybir.dt.uint16,
                         kind="ExternalInput").ap()
    out = nc.dram_tensor("out", [128, NC_PAD], BF16, kind="ExternalOutput").ap()
    with tile.TileContext(nc) as tc:
        kernel_body(tc, out, xa, xb, cpk)
    nc.compile()
    _CACHE["nc"] = nc
    return nc


def run(input, W, trace=False, **trace_kwargs):
    x_np = np.asarray(input, dtype=np.float32)
    w_np = np.asarray(W, dtype=np.float32)
    nc = build()
    in_maps = host_inputs(x_np, w_np)
    res = bass_utils.run_bass_kernel_spmd(
        nc, in_maps, core_ids=list(range(NCORES)), trace=trace, **trace_kwargs)
    return unshard(res), res


def kernel(input, W):
    out, _ = run(input, W, trace=False)
    return out
